# revision 8
# baseline (speedup 1.0000x reference)
"""AudioVisualBackbone TRN2 kernel — 8-core data-parallel (batch sharded).

Host: shards batch 64 -> 8 cores x 8, pre-transposes visual to [VD, L*S] per
batch and packs weights into partition-major SBUF layouts. Device (per core):
  f_vT = Wvp @ visual.T (+bvp)              [512, 784] per batch   (PE, f32r)
  score_vT = tanh(W_visual @ f_vT)          [512, 784] per batch   (PE+ACT)
  raw/colsum via one M=2 matmul against [score_a | ones]
  spatial softmax + temporal attention on small tiles, f_prime_v via
  scalar_tensor_tensor accumulation; audio path + cosine + gated fusion +
  classifier in a small epilogue. Per-batch cos goes to DRAM; the host takes
  mean(1-cos) over all 64 (the "all-reduce" of the similarity loss).
"""
import numpy as np

B, L, S = 64, 16, 49
VD, AD, D, H, C = 1028, 1024, 512, 256, 4
NCORES = 8
BL = B // NCORES          # local batch
R = L * S                 # 784 rows per batch
KV = 9                    # vd k-tiles: 8 full + remainder 4
KVR = VD - 8 * 128        # 4
JCH = ((0, 512), (512, 272))   # free-dim chunks of R
EPS = 1e-8

_cache = {}


def _build():
    import concourse.bacc as bacc
    import concourse.mybir as mybir
    import concourse.tile as tile
    import concourse.bass_isa as bass_isa
    from concourse.masks import make_identity

    F32 = mybir.dt.float32
    MMD = mybir.dt.float32r
    AF = mybir.ActivationFunctionType
    OP = mybir.AluOpType
    AX = mybir.AxisListType

    nc = bacc.Bacc("TRN2", target_bir_lowering=False, debug=False)

    # ---- DRAM I/O ----
    visualT_d = nc.dram_tensor("visualT", [BL, VD, R], MMD, kind="ExternalInput")
    audioT_d = nc.dram_tensor("audioT", [128, 8, BL], MMD, kind="ExternalInput")
    wvpT_d = nc.dram_tensor("wvpT", [128, 8, D], MMD, kind="ExternalInput")
    wvpR_d = nc.dram_tensor("wvpR", [KVR, D], MMD, kind="ExternalInput")
    wapT_d = nc.dram_tensor("wapT", [128, 8, D], MMD, kind="ExternalInput")
    wvisT_d = nc.dram_tensor("wvisT", [128, 4, D], MMD, kind="ExternalInput")
    waudT_d = nc.dram_tensor("waudT", [128, 4, D], MMD, kind="ExternalInput")
    uvT_d = nc.dram_tensor("uvT", [128, 4, D], MMD, kind="ExternalInput")
    uaT_d = nc.dram_tensor("uaT", [128, 4, D], MMD, kind="ExternalInput")
    wc1T_d = nc.dram_tensor("wc1T", [128, 4, H], MMD, kind="ExternalInput")
    wc2T_d = nc.dram_tensor("wc2T", [128, 2, C], MMD, kind="ExternalInput")
    bvp_d = nc.dram_tensor("bvp_t", [128, 4], F32, kind="ExternalInput")
    bap_d = nc.dram_tensor("bap_t", [128, 4], F32, kind="ExternalInput")
    bc1_d = nc.dram_tensor("bc1_t", [128, 2], F32, kind="ExternalInput")
    bc2_d = nc.dram_tensor("bc2_t", [C, 1], F32, kind="ExternalInput")
    logitsT_d = nc.dram_tensor("logitsT", [C, BL], F32, kind="ExternalOutput")
    cosv_d = nc.dram_tensor("cosv", [BL, 1], F32, kind="ExternalOutput")

    with tile.TileContext(nc) as tc:
        consts = tc.alloc_tile_pool(name="consts", bufs=1)
        vis_p = tc.alloc_tile_pool(name="vis", bufs=10)
        fv_p = tc.alloc_tile_pool(name="fv", bufs=2)
        sv_p = tc.alloc_tile_pool(name="sv", bufs=2)
        sm_p = tc.alloc_tile_pool(name="sm", bufs=2)
        ps_mm = tc.alloc_tile_pool(name="psmm", bufs=6, space="PSUM")
        ps_sm = tc.alloc_tile_pool(name="pssm", bufs=2, space="PSUM")

        ident = consts.tile([128, 128], F32)
        make_identity(nc, ident)

        # ---- load constants ----
        wvp_sb = consts.tile([128, 8, D], MMD)
        nc.sync.dma_start(out=wvp_sb, in_=wvpT_d[:])
        wvpR_sb = consts.tile([KVR, D], MMD)
        nc.sync.dma_start(out=wvpR_sb, in_=wvpR_d[:])
        wap_sb = consts.tile([128, 8, D], MMD)
        nc.sync.dma_start(out=wap_sb, in_=wapT_d[:])
        wvis_sb = consts.tile([128, 4, D], MMD)
        nc.sync.dma_start(out=wvis_sb, in_=wvisT_d[:])
        waud_sb = consts.tile([128, 4, D], MMD)
        nc.sync.dma_start(out=waud_sb, in_=waudT_d[:])
        uv_sb = consts.tile([128, 4, D], MMD)
        nc.sync.dma_start(out=uv_sb, in_=uvT_d[:])
        ua_sb = consts.tile([128, 4, D], MMD)
        nc.sync.dma_start(out=ua_sb, in_=uaT_d[:])
        wc1_sb = consts.tile([128, 4, H], MMD)
        nc.sync.dma_start(out=wc1_sb, in_=wc1T_d[:])
        wc2_sb = consts.tile([128, 2, C], MMD)
        nc.sync.dma_start(out=wc2_sb, in_=wc2T_d[:])
        audio_sb = consts.tile([128, 8, BL], MMD)
        nc.sync.dma_start(out=audio_sb, in_=audioT_d[:])
        bvp_sb = consts.tile([128, 4], F32)
        nc.sync.dma_start(out=bvp_sb, in_=bvp_d[:])
        bap_sb = consts.tile([128, 4], F32)
        nc.sync.dma_start(out=bap_sb, in_=bap_d[:])
        bc1_sb = consts.tile([128, 2], F32)
        nc.sync.dma_start(out=bc1_sb, in_=bc1_d[:])
        bc2_sb = consts.tile([C, 1], F32)
        nc.sync.dma_start(out=bc2_sb, in_=bc2_d[:])

        # persistent accumulators
        f_aT = consts.tile([128, 4, BL], MMD)
        score_aT = consts.tile([128, 4, BL, 2], MMD)   # [.., 0]=score_a, [.., 1]=1.0
        fpvT = consts.tile([128, 4, BL], MMD)
        avg_parts = consts.tile([128, 4, BL, 2], F32)

        nc.vector.memset(score_aT[:, :, :, 1].bitcast(F32), 1.0)  # ones column

        # ---- audio chain ----
        for m in range(4):
            ms = slice(m * 128, (m + 1) * 128)
            ps = ps_sm.tile([128, BL], F32, tag="sm")
            for kt in range(8):
                nc.tensor.matmul(ps, wap_sb[:, kt, ms], audio_sb[:, kt, :],
                                 start=(kt == 0), stop=(kt == 7))
            nc.scalar.activation(out=f_aT[:, m, :], in_=ps, func=AF.Identity,
                                 bias=bap_sb[:, m:m + 1], scale=1.0)
        for m in range(4):
            ms = slice(m * 128, (m + 1) * 128)
            ps = ps_sm.tile([128, BL], F32, tag="sm")
            for kt in range(4):
                nc.tensor.matmul(ps, waud_sb[:, kt, ms], f_aT[:, kt, :],
                                 start=(kt == 0), stop=(kt == 3))
            nc.scalar.activation(out=score_aT[:, m, :, 0], in_=ps, func=AF.Tanh)

        # ---- per-batch visual pipeline ----
        for b in range(BL):
            vis = []
            for kt in range(KV):
                t = vis_p.tile([128, R], MMD, tag="vis")
                kk = 128 if kt < 8 else KVR
                nc.sync.dma_start(out=t[0:kk, :],
                                  in_=visualT_d[b, kt * 128:kt * 128 + kk, :])
                vis.append((t, kk))

            f_vT = fv_p.tile([128, 4, R], MMD, tag="fv")
            score_vT = sv_p.tile([128, 4, R], MMD, tag="sv")
            rc_sb = sm_p.tile([2, R], F32, tag="rc")

            for j, (joff, jn) in enumerate(JCH):
                js = slice(joff, joff + jn)
                # M1: f_vT = WvpT.T @ visualT
                m1 = [ps_mm.tile([128, 512], F32, tag="mm", name=f"m1_{b}_{j}_{i}") for i in range(4)]
                for kt in range(KV):
                    t, kk = vis[kt]
                    for m in range(4):
                        lhs = (wvp_sb[0:kk, kt, m * 128:(m + 1) * 128] if kt < 8
                               else wvpR_sb[0:kk, m * 128:(m + 1) * 128])
                        nc.tensor.matmul(m1[m][:, 0:jn], lhs, t[0:kk, js],
                                         start=(kt == 0), stop=(kt == KV - 1))
                for m in range(4):
                    nc.scalar.activation(out=f_vT[:, m, js], in_=m1[m][:, 0:jn],
                                         func=AF.Identity,
                                         bias=bvp_sb[:, m:m + 1], scale=1.0)
                # M2: score_vT = tanh(WvisT.T @ f_vT), accum avg partials
                m2 = [ps_mm.tile([128, 512], F32, tag="mm", name=f"m2_{b}_{j}_{i}") for i in range(4)]
                for kt in range(4):
                    for m in range(4):
                        nc.tensor.matmul(m2[m][:, 0:jn],
                                         wvis_sb[:, kt, m * 128:(m + 1) * 128],
                                         f_vT[:, kt, js],
                                         start=(kt == 0), stop=(kt == 3))
                for m in range(4):
                    nc.scalar.activation(out=score_vT[:, m, js], in_=m2[m][:, 0:jn],
                                         func=AF.Tanh,
                                         accum_out=avg_parts[:, m, b, j:j + 1])
                # raw + colsum in one M=2 matmul: lhsT = [score_a_b | ones]
                rc = ps_sm.tile([2, 512], F32, tag="sm")
                for kt in range(4):
                    nc.tensor.matmul(rc[:, 0:jn], score_aT[:, kt, b, :],
                                     score_vT[:, kt, js],
                                     start=(kt == 0), stop=(kt == 3))
                nc.vector.tensor_copy(out=rc_sb[:, js], in_=rc[:, 0:jn])

            # ---- per-batch attention smalls ----
            rc_ls = sm_p.tile([16, 2, S], F32, tag="rcls")  # [:,0]=raw, [:,1]=colsum
            nc.sync.dma_start(out=rc_ls[:, 0, :],
                              in_=rc_sb[0:1, :].rearrange("o (l s) -> o l s", l=L))
            nc.sync.dma_start(out=rc_ls[:, 1, :],
                              in_=rc_sb[1:2, :].rearrange("o (l s) -> o l s", l=L))
            nmx = sm_p.tile([16, 1], F32, tag="nmx")
            nc.vector.reduce_max(out=nmx, in_=rc_ls[:, 0, :], axis=AX.X, negate=True)
            e_ls = sm_p.tile([16, S], F32, tag="els")
            s_e = sm_p.tile([16, 1], F32, tag="se")
            nc.scalar.activation(out=e_ls, in_=rc_ls[:, 0, :], func=AF.Exp,
                                 bias=nmx, scale=1.0, accum_out=s_e)
            scr16 = sm_p.tile([16, S], F32, tag="scr16")
            ecs = sm_p.tile([16, 1], F32, tag="ecs")
            nc.vector.scalar_tensor_tensor(out=scr16, in0=e_ls, scalar=1.0,
                                           in1=rc_ls[:, 1, :], op0=OP.mult,
                                           op1=OP.mult, accum_out=ecs)
            rcp_s = sm_p.tile([16, 1], F32, tag="rcps")
            nc.vector.reciprocal(out=rcp_s, in_=s_e)
            sws = sm_p.tile([16, 1], F32, tag="sws")
            nc.vector.tensor_mul(sws, ecs, rcp_s)
            # temporal softmax over l (partition dim, 16 channels)
            mx_t = sm_p.tile([16, 1], F32, tag="mxt")
            nc.gpsimd.partition_all_reduce(mx_t, sws, channels=16,
                                           reduce_op=bass_isa.ReduceOp.max)
            nmx_t = sm_p.tile([16, 1], F32, tag="nmxt")
            nc.vector.tensor_scalar_mul(out=nmx_t, in0=mx_t, scalar1=-1.0)
            e_t = sm_p.tile([16, 1], F32, tag="et")
            nc.scalar.activation(out=e_t, in_=sws, func=AF.Exp, bias=nmx_t, scale=1.0)
            s_t = sm_p.tile([16, 1], F32, tag="st")
            nc.gpsimd.partition_all_reduce(s_t, e_t, channels=16,
                                           reduce_op=bass_isa.ReduceOp.add)
            r_t = sm_p.tile([16, 1], F32, tag="rt")
            nc.vector.reciprocal(out=r_t, in_=s_t)
            c16 = sm_p.tile([16, 1], F32, tag="c16")
            nc.vector.scalar_tensor_tensor(out=c16, in0=e_t, scalar=r_t, in1=rcp_s,
                                           op0=OP.mult, op1=OP.mult)
            u_ls = sm_p.tile([16, S], F32, tag="uls")
            nc.vector.tensor_scalar_mul(out=u_ls, in0=e_ls, scalar1=c16)
            u_flat = sm_p.tile([1, R], F32, tag="uflat")
            nc.sync.dma_start(out=u_flat.rearrange("o (l s) -> o l s", l=L), in_=u_ls)
            u_bc = sm_p.tile([128, R], F32, tag="ubc")
            nc.gpsimd.partition_broadcast(u_bc, u_flat)
            for m in range(4):
                nc.vector.scalar_tensor_tensor(out=score_vT[:, m, :],
                                               in0=f_vT[:, m, :],
                                               scalar=1.0, in1=u_bc, op0=OP.mult,
                                               op1=OP.mult,
                                               accum_out=fpvT[:, m, b:b + 1])

        # ---- epilogue: audio attention, cosine, fusion, classifier ----
        avg_T = consts.tile([128, 4, BL], F32)
        nc.vector.reduce_sum(out=avg_T, in_=avg_parts, axis=AX.X)

        avg_rows = consts.tile([BL, D], F32)
        sa_rows = consts.tile([BL, D], F32)
        fa_rows = consts.tile([BL, D], F32)
        fpv_rows = consts.tile([BL, D], F32)
        for ti, (src, dst) in enumerate((
                (avg_T, avg_rows), (None, sa_rows),
                (f_aT, fa_rows), (fpvT, fpv_rows))):
            for m in range(4):
                tp = ps_mm.tile([BL, 128], F32, tag="mm", name=f"tp_{ti}_{m}")
                in_ = score_aT[:, m, :, 0] if src is None else src[:, m, :]
                if in_.dtype == MMD:
                    in_ = in_.bitcast(F32)
                nc.tensor.transpose(tp, in_, ident)
                nc.scalar.activation(out=dst[:, m * 128:(m + 1) * 128], in_=tp,
                                     func=AF.Copy)

        prod = consts.tile([BL, D], F32)
        nc.vector.scalar_tensor_tensor(out=prod, in0=avg_rows, scalar=1.0 / R,
                                       in1=sa_rows, op0=OP.mult, op1=OP.mult)
        nmx_a = consts.tile([BL, 1], F32)
        nc.vector.reduce_max(out=nmx_a, in_=prod, axis=AX.X, negate=True)
        e_a = consts.tile([BL, D], F32)
        s_a = consts.tile([BL, 1], F32)
        nc.scalar.activation(out=e_a, in_=prod, func=AF.Exp, bias=nmx_a,
                             scale=1.0, accum_out=s_a)
        rcp_a = consts.tile([BL, 1], F32)
        nc.vector.reciprocal(out=rcp_a, in_=s_a)
        fpa_rows = consts.tile([BL, D], F32)
        nc.vector.scalar_tensor_tensor(out=fpa_rows, in0=e_a, scalar=rcp_a,
                                       in1=fa_rows, op0=OP.mult, op1=OP.mult)

        # cosine similarity per local batch
        scr8 = consts.tile([BL, D], F32)
        nv2 = consts.tile([BL, 1], F32)
        na2 = consts.tile([BL, 1], F32)
        dot = consts.tile([BL, 1], F32)
        nc.scalar.activation(out=scr8, in_=fpv_rows, func=AF.Square, accum_out=nv2)
        nc.scalar.activation(out=scr8, in_=fpa_rows, func=AF.Square, accum_out=na2)
        nc.vector.scalar_tensor_tensor(out=scr8, in0=fpv_rows, scalar=1.0,
                                       in1=fpa_rows, op0=OP.mult, op1=OP.mult,
                                       accum_out=dot)
        nv = consts.tile([BL, 1], F32)
        nc.scalar.activation(out=nv, in_=nv2, func=AF.Sqrt)
        na = consts.tile([BL, 1], F32)
        nc.scalar.activation(out=na, in_=na2, func=AF.Sqrt)
        nc.vector.tensor_scalar_max(out=nv, in0=nv, scalar1=EPS)
        nc.vector.tensor_scalar_max(out=na, in0=na, scalar1=EPS)
        den = consts.tile([BL, 1], F32)
        nc.vector.tensor_mul(den, nv, na)
        rden = consts.tile([BL, 1], F32)
        nc.vector.reciprocal(out=rden, in_=den)
        cos_sb = consts.tile([BL, 1], F32)
        nc.vector.tensor_mul(cos_sb, dot, rden)
        nc.sync.dma_start(out=cosv_d[:], in_=cos_sb)

        # f_prime_a back to transposed layout
        fpaT = consts.tile([128, 4, BL], MMD)
        for m in range(4):
            tp = ps_mm.tile([128, BL], F32, tag="mm")
            nc.tensor.transpose(tp, fpa_rows[:, m * 128:(m + 1) * 128],
                                ident[0:BL, 0:BL])
            nc.scalar.activation(out=fpaT[:, m, :], in_=tp, func=AF.Copy)

        # gated fusion: fused = sigmoid(Uv @ fpv) + sigmoid(Ua @ fpa)
        fusedT = consts.tile([128, 4, BL], MMD)
        gv = consts.tile([128, 4, BL], F32)
        ga = consts.tile([128, 4, BL], F32)
        for m in range(4):
            ms = slice(m * 128, (m + 1) * 128)
            p1 = ps_mm.tile([128, BL], F32, tag="mm")
            for kt in range(4):
                nc.tensor.matmul(p1, uv_sb[:, kt, ms], fpvT[:, kt, :],
                                 start=(kt == 0), stop=(kt == 3))
            nc.scalar.activation(out=gv[:, m, :], in_=p1, func=AF.Sigmoid)
            p2 = ps_mm.tile([128, BL], F32, tag="mm")
            for kt in range(4):
                nc.tensor.matmul(p2, ua_sb[:, kt, ms], fpaT[:, kt, :],
                                 start=(kt == 0), stop=(kt == 3))
            nc.scalar.activation(out=ga[:, m, :], in_=p2, func=AF.Sigmoid)
        nc.vector.tensor_add(fusedT, gv, ga)

        # classifier
        hidT = consts.tile([128, 2, BL], MMD)
        for m in range(2):
            ph = ps_mm.tile([128, BL], F32, tag="mm")
            for kt in range(4):
                nc.tensor.matmul(ph, wc1_sb[:, kt, m * 128:(m + 1) * 128],
                                 fusedT[:, kt, :], start=(kt == 0), stop=(kt == 3))
            nc.scalar.activation(out=hidT[:, m, :], in_=ph, func=AF.Relu,
                                 bias=bc1_sb[:, m:m + 1], scale=1.0)
        pl = ps_sm.tile([C, BL], F32, tag="sm")
        for kt in range(2):
            nc.tensor.matmul(pl, wc2_sb[:, kt, :], hidT[:, kt, :],
                             start=(kt == 0), stop=(kt == 1))
        lg = consts.tile([C, BL], F32)
        nc.scalar.activation(out=lg, in_=pl, func=AF.Identity,
                             bias=bc2_sb, scale=1.0)
        nc.sync.dma_start(out=logitsT_d[:], in_=lg)

        ps_sm.release()
        ps_mm.release()
        sm_p.release()
        sv_p.release()
        fv_p.release()
        vis_p.release()
        consts.release()

    nc.compile()
    return nc


def _part_major(w2d, kt):
    """[kt*128, N] -> [128, kt, N] partition-major."""
    n = w2d.shape[1]
    return np.ascontiguousarray(
        w2d.reshape(kt, 128, n).transpose(1, 0, 2), dtype=np.float32)


def _prep_weights(Wvp, bvp, Wap, bap, W_audio, W_visual, U_visual, U_audio,
                  Wc1, bc1, Wc2, bc2):
    WvpT = np.ascontiguousarray(Wvp.T, np.float32)        # [1028, 512]
    return {
        "wvpT": _part_major(WvpT[0:1024], 8),
        "wvpR": np.ascontiguousarray(WvpT[1024:VD], np.float32),
        "wapT": _part_major(np.ascontiguousarray(Wap.T, np.float32), 8),
        "wvisT": _part_major(np.ascontiguousarray(W_visual.T, np.float32), 4),
        "waudT": _part_major(np.ascontiguousarray(W_audio.T, np.float32), 4),
        "uvT": _part_major(np.ascontiguousarray(U_visual.T, np.float32), 4),
        "uaT": _part_major(np.ascontiguousarray(U_audio.T, np.float32), 4),
        "wc1T": _part_major(np.ascontiguousarray(Wc1.T, np.float32), 4),
        "wc2T": _part_major(np.ascontiguousarray(Wc2.T, np.float32), 2),
        "bvp_t": np.ascontiguousarray(bvp.reshape(4, 128).T, np.float32),
        "bap_t": np.ascontiguousarray(bap.reshape(4, 128).T, np.float32),
        "bc1_t": np.ascontiguousarray(bc1.reshape(2, 128).T, np.float32),
        "bc2_t": np.ascontiguousarray(bc2.reshape(C, 1), np.float32),
    }


def run(inputs, trace=False):
    from concourse.bass_utils import run_bass_kernel_spmd

    if "nc" not in _cache:
        _cache["nc"] = _build()
    nc = _cache["nc"]

    visual = np.asarray(inputs["visual"], np.float32)
    audio = np.asarray(inputs["audio"], np.float32)
    wmap = _prep_weights(
        np.asarray(inputs["Wvp"], np.float32), np.asarray(inputs["bvp"], np.float32),
        np.asarray(inputs["Wap"], np.float32), np.asarray(inputs["bap"], np.float32),
        np.asarray(inputs["W_audio"], np.float32),
        np.asarray(inputs["W_visual"], np.float32),
        np.asarray(inputs["U_visual"], np.float32),
        np.asarray(inputs["U_audio"], np.float32),
        np.asarray(inputs["Wc1"], np.float32), np.asarray(inputs["bc1"], np.float32),
        np.asarray(inputs["Wc2"], np.float32), np.asarray(inputs["bc2"], np.float32))

    vis_r = visual.reshape(NCORES, BL, R, VD)
    aud_r = audio.reshape(NCORES, BL, AD)
    in_maps = []
    for c in range(NCORES):
        m = dict(wmap)
        m["visualT"] = np.ascontiguousarray(
            vis_r[c].transpose(0, 2, 1), np.float32)          # [BL, VD, R]
        m["audioT"] = np.ascontiguousarray(
            aud_r[c].T.reshape(8, 128, BL).transpose(1, 0, 2), np.float32)
        in_maps.append(m)

    res = run_bass_kernel_spmd(nc, in_maps, list(range(NCORES)), trace=trace)
    logits = np.concatenate(
        [res.results[c]["logitsT"].T for c in range(NCORES)], axis=0)
    cos = np.concatenate(
        [res.results[c]["cosv"][:, 0] for c in range(NCORES)], axis=0)
    loss = np.float32(np.mean(1.0 - cos.astype(np.float64)))
    return (np.ascontiguousarray(logits, np.float32), loss), res


def kernel(**inputs):
    out, _ = run(inputs)
    return out
